# revision 1
# baseline (speedup 1.0000x reference)
"""CARAFE content-aware upsampling (S=2, K=5) as a Trainium2 Bass/Tile kernel.

Sharding: 8 cores = 2 batches x 4 row-quarters (16 low-res rows each).
Per-core pipeline (all compute on device):
  1. content encoder 1x1 conv (PE, f32r):       xc[64, 20*68]
  2. kernel predictor 3x3 conv (PE, f32r):      kp[100, 16*64]   (9 shifted matmuls)
  3. softmax over k2 (no max-sub; ACT exp + PE selector-sums + DVE recip/mult)
  4. per 2x16-position tile (32 tiles):
     a. PE transpose of pq-replicated prob columns -> PT[128=(pos,pq), 100]
     b. GPSIMD local_scatter (static per-partition idx) -> M^T[128, 120]
     c. PE transpose -> M[120 patch, 128 (pos,pq)]
     d. PE matmul  out[128 (pos,pq), 256 c] = M^T.T?? = M.T @ patch  (1 matmul)
  5. DMA out in (tile, pos, pq)-major layout; host reassembles (untimed).

Host prep (untimed): spatial zero-pad + slicing, pre-built patch tiles
(pure x data relayout), weight transposes, static index/selector tables.
"""

import os

os.environ.setdefault("MYCRO_LOCAL_CACHE", "1")

import numpy as np

import ml_dtypes
import concourse.bacc as bacc
import concourse.mybir as mybir
import concourse.tile as tile
from concourse.bass_utils import run_bass_kernel_spmd

F32 = mybir.dt.float32
F32R = mybir.dt.float32r
BF16 = mybir.dt.bfloat16
I16 = mybir.dt.int16
U8 = mybir.dt.uint8
AF = mybir.ActivationFunctionType

B, C, H, W = 2, 256, 64, 64
S, K, COMP = 2, 5, 64
KP = 100          # S*S*K*K
K2 = 25
NCORES = 8
ROWS = 16         # low-res rows per core
RP, WPAD = 20, 68  # padded slice rows/cols
NTH, NTW = 8, 4   # tile grid: 8 x 4 tiles of 2x16 positions
NT = NTH * NTW    # 32 tiles
TPH, TPW = 2, 16  # tile position grid
POS = TPH * TPW   # 32
PATCH = 120       # (TPW+4) * (TPH+4) = 20*6, index = ww*6 + hh
GRID = RP * WPAD  # 1360
HWF = ROWS * W    # 1024 (conv-output free size)


def _static_tables():
    # local_scatter index table: partition m = 4*(h'*16+w') + pq,
    # data col ch = 4*k2 + pq'; scatter to patch idx (w'+b)*6 + (h'+a)
    # only when pq' == pq(m).
    idx = np.full((128, 4 * KP), -1, dtype=np.int16)
    for m in range(128):
        pq, pl = m // POS, m % POS
        hp, wp = pl // TPW, pl % TPW
        for k2 in range(K2):
            a, b = k2 // K, k2 % K
            p = (wp + b) * 6 + (hp + a)
            for j in range(4):
                idx[m, j * KP + 4 * k2 + pq] = j * PATCH + p
    selT = np.zeros((KP, 4), dtype=np.float32)
    for ch in range(KP):
        selT[ch, ch % 4] = 1.0
    ident = np.eye(128, dtype=np.float32).astype(ml_dtypes.bfloat16)
    return idx, selT, selT.T.copy(), ident


def build_kernel():
    nc = bacc.Bacc("TRN2", target_bir_lowering=False, debug=False)

    xs_c = nc.dram_tensor("xs_c", [C, GRID], F32R, kind="ExternalInput").ap()
    patches_d = nc.dram_tensor(
        "patches", [PATCH, NT * C], BF16, kind="ExternalInput"
    ).ap()
    w_kp9 = nc.dram_tensor("w_kp9", [COMP, 9 * KP], F32R, kind="ExternalInput").ap()
    blobf_d = nc.dram_tensor("blobf", [128, 232], F32R, kind="ExternalInput").ap()
    blob_d = nc.dram_tensor("blob", [128, 1064], U8, kind="ExternalInput").ap()
    out_d = nc.dram_tensor("out", [128, NT * C], BF16, kind="ExternalOutput").ap()

    with tile.TileContext(nc) as tc:
        _build(tc, nc, xs_c, patches_d, w_kp9, blobf_d, blob_d, out_d)
    nc.compile()
    return nc


def _build(tc, nc, xs_c, patches_d, w_kp9, blobf_d, blob_d, out_d):
    with (
        tc.tile_pool(name="const", bufs=1) as cpool,
        tc.tile_pool(name="work", bufs=1) as wpool,
        tc.tile_pool(name="ptile", bufs=4) as ppool,
        tc.tile_pool(name="opool", bufs=2) as opool,
        tc.tile_pool(name="ps_big", bufs=2, space="PSUM") as ps_big,
        tc.tile_pool(name="ps_pt", bufs=2, space="PSUM") as ps_pt,
        tc.tile_pool(name="ps_m", bufs=2, space="PSUM") as ps_m,
        tc.tile_pool(name="ps_o", bufs=2, space="PSUM") as ps_o,
    ):
        # ---- constant loads (one packed blob + x + wkp + patches)
        x_sb = cpool.tile([128, 2, GRID], F32R, tag="x")
        nc.sync.dma_start(x_sb[:], xs_c.rearrange("(blk p) f -> p blk f", p=128))
        blobf_sb = cpool.tile([128, 232], F32R, tag="blobf")
        nc.sync.dma_start(blobf_sb[:], blobf_d)
        wkp_sb = cpool.tile([COMP, 9 * KP], F32R, tag="wkp")
        nc.sync.dma_start(wkp_sb[:], w_kp9)
        wenc_sb = blobf_sb[:, 0:128].rearrange("p (blk m) -> p blk m", blk=2)
        selT_sb = blobf_sb[0:KP, 128:132]
        selTT_sb = blobf_sb[0:4, 132:232]
        blob_sb = cpool.tile([128, 1064], U8, tag="blob")
        nc.sync.dma_start(blob_sb[:], blob_d)
        ident_sb = blob_sb[:, 0:256].bitcast(BF16)
        idx4_sb = blob_sb[:, 256:1056].bitcast(I16)
        benc_sb = blob_sb[0:COMP, 1056:1060].bitcast(F32)
        bkp_sb = blob_sb[0:KP, 1060:1064].bitcast(F32)
        pat_sb = cpool.tile([PATCH, NT * C], BF16, tag="pat")
        nc.sync.dma_start(pat_sb[:], patches_d)

        # ---- phase 1: encoder 1x1 conv -> xc [64, GRID] (padded grid)
        xc_sb = wpool.tile([COMP, GRID], F32R, tag="xc")
        for lo in range(0, GRID, 512):
            hi = min(lo + 512, GRID)
            xc_ps = ps_big.tile([COMP, hi - lo], F32, tag="big")
            for blk in range(2):
                nc.tensor.matmul(
                    xc_ps[:],
                    wenc_sb[:, blk, :],
                    x_sb[:, blk, lo:hi],
                    start=(blk == 0), stop=(blk == 1),
                )
            nc.scalar.activation(
                xc_sb[:, lo:hi], xc_ps[:], AF.Identity, bias=benc_sb
            )

        # ---- phases 2+3, per 8-row chunk: conv -> exp -> Z -> recip -> rep -> mul
        xc_g = xc_sb[:].rearrange("p (r w) -> p r w", r=RP)
        expP_sb = wpool.tile([KP, HWF], F32R, tag="expP")
        rz_sb = wpool.tile([4, HWF], F32R, tag="rz")
        p_sb = wpool.tile([KP, HWF], BF16, tag="P")
        pt_g = p_sb[:].rearrange("p (t l) -> p t l", t=NT)
        for ch in range(2):
            sl = slice(512 * ch, 512 * (ch + 1))
            r0 = 1 + 8 * ch
            kp_ps = ps_big.tile([KP, 512], F32, tag="big", name="kp_ps")
            for tap in range(9):
                ti, tj = tap // 3, tap % 3
                rhs = xc_g[:, r0 + ti: r0 + ti + 8, 1 + tj: 65 + tj]
                nc.tensor.matmul(
                    kp_ps[:],
                    wkp_sb[:, tap * KP:(tap + 1) * KP],
                    rhs,
                    start=(tap == 0), stop=(tap == 8),
                )
            nc.scalar.activation(expP_sb[:, sl], kp_ps[:], AF.Exp, bias=bkp_sb)
            z_ps = ps_big.tile([4, 512], F32, tag="big", name="z_ps")
            nc.tensor.matmul(
                z_ps[:], selT_sb, expP_sb[:, sl], start=True, stop=True)
            with nc.allow_low_precision(reason="recip feeds f32r matmul"):
                nc.vector.reciprocal(rz_sb[:, sl], z_ps[:])
            rep_ps = ps_big.tile([KP, 512], F32, tag="big", name="rep_ps")
            nc.tensor.matmul(
                rep_ps[:], selTT_sb, rz_sb[:, sl], start=True, stop=True)
            # normalize + reorder TILE-MAJOR within this chunk's 16 tiles:
            # P col = t*32 + hp*16 + w', t = (4*ch + thl)*4 + tw
            ep_c = expP_sb[:, sl].rearrange("p (r w) -> p r w", r=8)
            rp_c = rep_ps[:].rearrange("p (r w) -> p r w", r=8)
            for hp in range(TPH):
                src_e = ep_c[:, hp::2, :].rearrange(
                    "p thl (tw w) -> p thl tw w", tw=NTW)
                src_r = rp_c[:, hp::2, :].rearrange(
                    "p thl (tw w) -> p thl tw w", tw=NTW)
                dst = pt_g[:, 16 * ch: 16 * (ch + 1),
                           TPW * hp: TPW * (hp + 1)].rearrange(
                    "p (thl tw) w -> p thl tw w", tw=NTW)
                nc.vector.tensor_mul(dst, src_e, src_r)
        # pq-replicate tile-major per chunk: Prep col = tl*128 + pq*32 + pl
        preps = []
        for ch in range(2):
            prep_sb = wpool.tile([KP, 16 * 128], BF16, tag="Prep",
                                 name=f"prep{ch}_sb", bufs=2)
            rep_src = p_sb[:, 512 * ch: 512 * (ch + 1)].rearrange(
                "p (t l) -> p t l", t=16)
            rep_src = rep_src.unsqueeze(2).broadcast_to([KP, 16, 4, POS])
            if ch == 0:
                nc.vector.tensor_copy(prep_sb[:], rep_src)
            else:
                nc.scalar.copy(prep_sb[:], rep_src)
            preps.append(prep_sb)

        # ---- phase 4: reassembly, tile-pairs, 2-stage software pipeline
        NPAIR = NT // 2
        LAG = 3
        mts = [None] * NPAIR
        ost = [None]

        def stage_a(u):
            t0 = 2 * u
            pt_ps = ps_pt.tile([128, 2 * KP], BF16, tag="pt")
            for j in range(2):
                t = t0 + j
                nc.tensor.transpose(
                    pt_ps[:, KP * j: KP * (j + 1)],
                    preps[t // 16][:, 128 * (t % 16): 128 * (t % 16 + 1)],
                    ident_sb[0:KP, 0:KP])
            ptr_sb = ppool.tile([128, 2 * KP], BF16, tag="ptr")
            if u % 2 == 0:
                nc.scalar.copy(ptr_sb[:], pt_ps[:])
            else:
                nc.vector.tensor_copy(ptr_sb[:], pt_ps[:])
            mt_sb = ppool.tile([128, 2 * PATCH], BF16, tag="mt")
            nc.gpsimd.local_scatter(
                mt_sb[:], ptr_sb[:], idx4_sb[:, 0: 2 * KP],
                channels=128, num_elems=2 * PATCH, num_idxs=2 * KP,
            )
            mts[u] = mt_sb

        def stage_b(u):
            t0 = 2 * u
            mt_sb = mts[u]
            if u % 4 == 0:
                ost[0] = opool.tile([128, 8 * C], BF16, tag="ost", name="ost_t")
            o_ps = ps_o.tile([128, 2 * C], F32, tag="o")
            for j in range(2):
                m_ps = ps_m.tile([PATCH, 128], BF16, tag="m")
                nc.tensor.transpose(
                    m_ps[:], mt_sb[:, PATCH * j: PATCH * (j + 1)], ident_sb)
                m_sb = ppool.tile([PATCH, 128], BF16, tag="msb")
                if j == 0:
                    nc.vector.tensor_copy(m_sb[:], m_ps[:])
                else:
                    nc.scalar.copy(m_sb[:], m_ps[:])
                nc.tensor.matmul(
                    o_ps[:, C * j: C * (j + 1)], m_sb[:],
                    pat_sb[:, (t0 + j) * C:(t0 + j + 1) * C],
                    start=True, stop=True)
            w = (u % 4) * 2 * C
            if u % 2 == 0:
                nc.vector.tensor_copy(ost[0][:, w: w + 2 * C], o_ps[:])
            else:
                nc.scalar.copy(ost[0][:, w: w + 2 * C], o_ps[:])
            if u % 4 == 3:
                g = (u - 3) * 2 * C
                eng = nc.sync if (u // 4) % 2 == 0 else nc.scalar
                eng.dma_start(out_d[:, g: g + 8 * C], ost[0][:])

        for u in range(NPAIR + LAG):
            if u < NPAIR:
                stage_a(u)
            if u >= LAG:
                stage_b(u - LAG)


def host_prep(x, w_enc, b_enc, w_kp, b_kp):
    """Build per-core input maps (pure relayout, untimed)."""
    idx, selT, selTT, ident = _static_tables()
    xpad = np.pad(x, ((0, 0), (0, 0), (2, 2), (2, 2)))  # [B, C, 68, 68]
    w_encT = np.ascontiguousarray(w_enc.T)              # [256, 64]
    w_kp9 = np.ascontiguousarray(
        np.transpose(w_kp, (1, 2, 3, 0)).reshape(COMP, 9 * KP)
    )
    blobf = np.zeros((128, 232), np.float32)
    blobf[:, 0:128] = w_encT.reshape(2, 128, 64).transpose(1, 0, 2).reshape(128, 128)
    blobf[0:KP, 128:132] = selT
    blobf[0:4, 132:232] = selTT
    blob = np.zeros((128, 1064), np.uint8)
    blob[:, 0:256] = ident.view(np.uint8).reshape(128, 256)
    blob[:, 256:1056] = idx.view(np.uint8).reshape(128, 800)
    blob[0:COMP, 1056:1060] = np.ascontiguousarray(
        b_enc.reshape(COMP, 1)).view(np.uint8).reshape(COMP, 4)
    blob[0:KP, 1060:1064] = np.ascontiguousarray(
        b_kp.reshape(KP, 1)).view(np.uint8).reshape(KP, 4)

    in_maps = []
    for core in range(NCORES):
        b, q = core // 4, core % 4
        sl = xpad[b, :, 16 * q: 16 * q + RP, :]          # [C, 20, 68]
        xs_c = np.ascontiguousarray(sl.reshape(C, GRID), dtype=np.float32)
        # patch tiles: [PATCH, NT, C], p = ww*6 + hh  (p-major for one big DMA)
        pat = np.empty((NT, PATCH, C), dtype=ml_dtypes.bfloat16)
        for t in range(NT):
            th, tw = t // NTW, t % NTW
            blk = sl[:, 2 * th: 2 * th + 6, TPW * tw: TPW * tw + 20]  # [C,6,20]
            pat[t] = np.transpose(blk, (2, 1, 0)).reshape(PATCH, C)
        pat = np.ascontiguousarray(np.transpose(pat, (1, 0, 2)))
        in_maps.append({
            "xs_c": xs_c,
            "patches": pat.reshape(PATCH, NT * C),
            "w_kp9": w_kp9,
            "blobf": blobf,
            "blob": blob,
        })
    return in_maps


def host_assemble(results):
    """results: list of 8 dicts with 'out' [NT*128, C] -> full [B, C, 128, 128]."""
    out = np.empty((B, C, H * S, W * S), dtype=np.float32)
    for core in range(NCORES):
        b, q = core // 4, core % 4
        # out rows j = (2p+q)*32 + h'*16 + w', cols (t, c)
        a = results[core]["out"].astype(np.float32).reshape(2, 2, TPH, TPW, NTH, NTW, C)
        # dims: p, q2, h', w', th, tw, c -> [c, th, h', p, tw, w', q2]
        o = np.transpose(a, (6, 4, 2, 0, 5, 3, 1)).reshape(C, 32, 128)
        out[b, :, 32 * q: 32 * (q + 1), :] = o
    return out


_NC_CACHE = None


def kernel(x, w_enc, b_enc, w_kp, b_kp):
    global _NC_CACHE
    x = np.asarray(x)
    w_enc = np.asarray(w_enc)
    b_enc = np.asarray(b_enc)
    w_kp = np.asarray(w_kp)
    b_kp = np.asarray(b_kp)
    if _NC_CACHE is None:
        _NC_CACHE = build_kernel()
    nc = _NC_CACHE
    in_maps = host_prep(x, w_enc, b_enc, w_kp, b_kp)
    trace = os.environ.get("CARAFE_TRACE", "0") == "1"
    res = run_bass_kernel_spmd(nc, in_maps, list(range(NCORES)), trace=trace)
    out = host_assemble(res.results)
    if trace:
        kernel.last_exec_time_ns = res.exec_time_ns
        kernel.last_results = res
    return out



# revision 27
# speedup vs baseline: 1.5885x; 1.5885x over previous
"""CARAFE content-aware upsampling (S=2, K=5) as a Trainium2 Bass/Tile kernel.

Sharding: 8 cores = 2 batches x 4 row-quarters (16 low-res rows each).
Per-core pipeline (4-stage software pipeline over 4 conv chunks):
  A(ch): encoder-fed 3x3 kernel-predictor conv chunk (4 lo-res rows,
         9 accumulating f32r matmuls) -> exp (ACT, tile-major reorder)
  B(ch): per 4-tile group (128 positions): PE transpose of exp probs
         -> [pos, 100ch]; DVE k2-sum + reciprocal + normalize-mul;
         GPSIMD local_scatter -> M^T [pos, 4pq x 120patch]
  C(ch): 8 PE transposes -> M [120, (gl,pq,pos)]; DVE copy to SBUF
  D(ch): per tile: 1 matmul out[(pq,pos), 256c] = M.T @ patches;
         PSUM->SBUF bf16 copies (ACT/DVE/Pool); SP DMA out

Host prep (untimed): spatial zero-pad + slicing, pre-built patch tiles
(pure x data relayout), weight transposes, static index tables.
"""

import os

os.environ.setdefault("MYCRO_LOCAL_CACHE", "1")

import numpy as np

import ml_dtypes
import concourse.bacc as bacc
import concourse.mybir as mybir
import concourse.tile as tile
from concourse.bass_utils import run_bass_kernel_spmd

F32 = mybir.dt.float32
F32R = mybir.dt.float32r
BF16 = mybir.dt.bfloat16
I16 = mybir.dt.int16
U8 = mybir.dt.uint8
AF = mybir.ActivationFunctionType
AX = mybir.AxisListType
ALU = mybir.AluOpType

B, C, H, W = 2, 256, 64, 64
S, K, COMP = 2, 5, 64
KP = 100          # S*S*K*K
K2 = 25
NCORES = 8
ROWS = 16         # low-res rows per core
RP, WPAD = 20, 68  # padded slice rows/cols
NTH, NTW = 8, 4   # tile grid: 8 x 4 tiles of 2x16 positions
NT = NTH * NTW    # 32 tiles
TPH, TPW = 2, 16  # tile position grid
POS = TPH * TPW   # 32
PATCH = 120       # (TPW+4) * (TPH+4) = 20*6, index = ww*6 + hh
GRID = RP * WPAD  # 1360
NCH = 4           # conv chunks (4 lo-res rows each)
NG = 8            # 4-tile groups (128 positions each)


def _static_tables():
    # local_scatter index table: partition m = tw*32 + hp*16 + wp
    # (pos within a 4-tile group), data col ch = 4*k2 + pq; scatter to
    # pq*120 + patch idx p = (wp+b)*6 + (hp+a) for k2 = (a, b).
    idx = np.zeros((128, KP), dtype=np.int16)
    for m in range(128):
        hp, wp = (m % POS) // TPW, m % TPW
        for k2 in range(K2):
            a, b = k2 // K, k2 % K
            p = (wp + b) * 6 + (hp + a)
            for pq in range(4):
                idx[m, 4 * k2 + pq] = pq * PATCH + p
    ident = np.eye(128, dtype=np.float32).astype(ml_dtypes.bfloat16)
    return idx, ident


def build_kernel():
    nc = bacc.Bacc("TRN2", target_bir_lowering=False, debug=False)

    xs_c = nc.dram_tensor("xs_c", [C, GRID], BF16, kind="ExternalInput").ap()
    patches_d = nc.dram_tensor(
        "patches", [PATCH, NT * C], BF16, kind="ExternalInput"
    ).ap()
    w_kp9 = nc.dram_tensor("w_kp9", [COMP, 9 * KP], F32R, kind="ExternalInput").ap()
    blob_d = nc.dram_tensor("blob", [128, 720], U8, kind="ExternalInput").ap()
    out_d = nc.dram_tensor("out", [128, NT * C], BF16, kind="ExternalOutput").ap()

    with tile.TileContext(nc) as tc:
        _build(tc, nc, xs_c, patches_d, w_kp9, blob_d, out_d)
    nc.compile()
    return nc


def _build(tc, nc, xs_c, patches_d, w_kp9, blob_d, out_d):
    with (
        tc.tile_pool(name="const", bufs=1) as cpool,
        tc.tile_pool(name="zpool", bufs=2) as zpool,
        tc.tile_pool(name="wpool", bufs=3) as wpool,
        tc.tile_pool(name="mtpool", bufs=4) as mtpool,
        tc.tile_pool(name="mpool", bufs=3) as mpool,
        tc.tile_pool(name="opool", bufs=4) as opool,
        tc.tile_pool(name="ps_kp", bufs=1, space="PSUM") as ps_kp,
        tc.tile_pool(name="ps_pt", bufs=1, space="PSUM") as ps_pt,
        tc.tile_pool(name="ps_m", bufs=2, space="PSUM") as ps_m,
        tc.tile_pool(name="ps_o", bufs=2, space="PSUM") as ps_o,
    ):
        # ---- constant tiles + input DMAs (all issued on SP, need-ordered)
        x_sb = cpool.tile([128, 2, GRID], BF16, tag="x")
        wkp_sb = cpool.tile([COMP, 9 * KP], F32R, tag="wkp")
        blob_sb = cpool.tile([128, 720], U8, tag="blob")
        pat_sb = cpool.tile([PATCH, NT * C], BF16, tag="pat")
        xc_sb = cpool.tile([COMP, GRID], F32R, tag="xc")
        pexp_sb = cpool.tile([KP, ROWS * W], BF16, tag="pexp")

        xr = xs_c.rearrange("(blk p) f -> p blk f", p=128)
        nc.sync.dma_start(blob_sb[:], blob_d)
        nc.sync.dma_start(x_sb[:, :, 0:256], xr[:, :, 0:256])
        nc.sync.dma_start(x_sb[:, :, 256:512], xr[:, :, 256:512])
        nc.sync.dma_start(x_sb[:, :, 512:768], xr[:, :, 512:768])
        nc.sync.dma_start(wkp_sb[:], w_kp9)
        nc.sync.dma_start(x_sb[:, :, 768:GRID], xr[:, :, 768:GRID])
        for k in range(4):
            nc.sync.dma_start(
                pat_sb[:, 2048 * k: 2048 * (k + 1)],
                patches_d[:, 2048 * k: 2048 * (k + 1)],
            )

        wenc_sb = blob_sb[:, 0:256].bitcast(BF16).rearrange(
            "p (blk m) -> p blk m", blk=2)
        benc_sb = blob_sb[0:COMP, 256:260].bitcast(F32)
        bkp_sb = blob_sb[0:KP, 260:264].bitcast(F32)
        ident_sb = blob_sb[:, 264:520].bitcast(BF16)
        idx_sb = blob_sb[:, 520:720].bitcast(I16)

        # ---- phase 1: encoder 1x1 conv -> xc [64, GRID] (padded grid)
        # (psum via the ps_o pool's tag, which stage D reuses much later;
        #  bias-add + f32r copy on DVE, keeping ACT free for exp(0))
        for lo, hi in ((0, 256), (256, 512), (512, 768), (768, 1280),
                       (1280, GRID)):
            ph1_t = ps_o.tile([128, 1024], F32, tag="o", name="ph1_t")
            ph1_ps = ph1_t[0:COMP, 0:hi - lo]
            for blk in range(2):
                nc.tensor.matmul(
                    ph1_ps,
                    wenc_sb[:, blk, :],
                    x_sb[:, blk, lo:hi],
                    start=(blk == 0), stop=(blk == 1),
                )
            nc.vector.tensor_scalar_add(xc_sb[:, lo:hi], ph1_ps, benc_sb)

        xc_g = xc_sb[:].rearrange("p (r w) -> p r w", r=RP)

        # ---- 7-stage pipeline, one cross-engine hop per stage boundary
        mts = [None] * NG
        msbs = [None] * NG
        ptrs = [None] * NCH
        mps = [None] * NG
        ost_eng = [nc.scalar.copy, nc.vector.tensor_copy,
                   nc.scalar.copy, nc.gpsimd.tensor_copy]
        ost_ctr = [0]

        def stage_a(ch):
            kp_t = ps_kp.tile([128, 256], F32, tag="kp", name="kp_t")
            kp_ps = kp_t[0:KP, :]
            r0 = 1 + 4 * ch
            for tap in range(9):
                ti, tj = tap // 3, tap % 3
                nc.tensor.matmul(
                    kp_ps,
                    wkp_sb[:, tap * KP:(tap + 1) * KP],
                    xc_g[:, r0 + ti: r0 + ti + 4, 1 + tj: 65 + tj],
                    start=(tap == 0), stop=(tap == 8),
                )
            # exp with tile-major column reorder: col g*128 + tw*32 + hp*16 + wp
            # (two ops: ISA allows at most 3 free dims per AP)
            for gl in range(2):
                g = 2 * ch + gl
                ov = pexp_sb[:, 128 * g: 128 * (g + 1)].rearrange(
                    "p (tw hp wp) -> p hp tw wp", tw=4, hp=2, wp=16)
                iv = kp_ps[:, 128 * gl: 128 * (gl + 1)].rearrange(
                    "p (hp tw wp) -> p hp tw wp", hp=2, tw=4, wp=16)
                nc.scalar.activation(ov, iv, AF.Exp, bias=bkp_sb)

        def stage_b1(ch):
            # both groups of the chunk in single merged DVE ops
            pt_t = ps_pt.tile([128, 128], F32, tag="pt", name="pt_t")
            pt_ps = pt_t.bitcast(BF16)[:, 0:2 * KP]
            for gl in range(2):
                g = 2 * ch + gl
                nc.tensor.transpose(
                    pt_ps[:, KP * gl: KP * (gl + 1)],
                    pexp_sb[:, 128 * g: 128 * (g + 1)],
                    ident_sb[0:KP, 0:KP])
            z = zpool.tile([128, 8], F32, tag="z")
            nc.vector.tensor_reduce(
                z[:], pt_ps.rearrange("p (gl k q) -> p gl q k", gl=2, k=K2),
                axis=AX.X, op=ALU.add)
            rz = zpool.tile([128, 8], BF16, tag="rz")
            with nc.allow_low_precision(reason="probs stored bf16 anyway"):
                nc.vector.reciprocal(rz[:], z[:])
            ptr_sb = wpool.tile([128, 2 * KP], BF16, tag="ptr")
            nc.vector.tensor_mul(
                ptr_sb[:].rearrange("p (gl k q) -> p gl k q", gl=2, k=K2),
                pt_ps.rearrange("p (gl k q) -> p gl k q", gl=2, k=K2),
                rz[:].rearrange("p (gl q) -> p gl q", gl=2).unsqueeze(2)
                .broadcast_to([128, 2, K2, 4]))
            ptrs[ch] = ptr_sb

        def stage_b2(g):
            mt_sb = mtpool.tile([128, 4 * PATCH], BF16, tag="mt")
            nc.gpsimd.local_scatter(
                mt_sb[:], ptrs[g // 2][:, KP * (g % 2): KP * (g % 2 + 1)],
                idx_sb[:],
                channels=128, num_elems=4 * PATCH, num_idxs=KP,
            )
            mts[g] = mt_sb

        def stage_c1(g):
            m_ps = ps_m.tile([PATCH, 512], BF16, tag="m")
            mt_sb = mts[g]
            for pq in range(4):
                nc.tensor.transpose(
                    m_ps[:, 128 * pq: 128 * (pq + 1)],
                    mt_sb[:, PATCH * pq: PATCH * (pq + 1)],
                    ident_sb)
            mps[g] = m_ps

        def stage_c2(g):
            m_sb = mpool.tile([PATCH, 512], BF16, tag="msb")
            # DVE is the cheapest copier but is loaded during the conv
            # stream; ACT is free during the drain.
            # reorder (pq, tile, pos) -> (tile, pq, pos) during the copy so
            # each tile's lhsT slice is contiguous (HW: stationary matmul
            # APs must have a single free dim)
            dst = m_sb[:].rearrange("p (tw q m) -> p q tw m", tw=4, q=4)
            srcv = mps[g][:].rearrange("p (q tw m) -> p q tw m", q=4, tw=4)
            if g < 4:
                nc.vector.tensor_copy(dst, srcv)
            else:
                nc.scalar.copy(dst, srcv)
            msbs[g] = m_sb

        # GPSIMD cannot read PSUM on hardware: ACT/DVE only here.
        OST_ENG = [("a", "d"), ("a", "d"), ("d", "a"), ("a", "d"),
                   ("a", "d"), ("d", "a"), ("a", "d"), ("d", "a")]

        def stage_d(g):
            o_ps = ps_o.tile([128, 1024], F32, tag="o", name="o_ps")
            for tw in range(4):
                t = 4 * g + tw
                nc.tensor.matmul(
                    o_ps[:, 256 * tw: 256 * (tw + 1)],
                    msbs[g][:, 128 * tw: 128 * (tw + 1)],
                    pat_sb[:, C * t: C * (t + 1)],
                    start=True, stop=True)
            ost = opool.tile([128, 1024], BF16, tag="ost")
            eng_map = {"a": nc.scalar.copy, "d": nc.vector.tensor_copy,
                       "p": nc.gpsimd.tensor_copy}
            for jj in range(2):
                eng_map[OST_ENG[g][jj]](
                    ost[:, 512 * jj: 512 * (jj + 1)],
                    o_ps[:, 512 * jj: 512 * (jj + 1)])
            nc.sync.dma_start(out_d[:, 1024 * g: 1024 * (g + 1)], ost[:])

        # Schedule (git -> stage instance), issue order = oldest deps first:
        #   a(ch)@2ch  b1(ch)@2ch+1  b2(g)@g+3  c1(g)@g+4  c2(g)@g+5  d(g)@g+6
        for git in range(NG + 6):
            if 0 <= git - 5 < NG:
                stage_c2(git - 5)
            if 0 <= git - 6 < NG:
                stage_d(git - 6)
            if 0 <= git - 3 < NG:
                stage_b2(git - 3)
            if git % 2 == 0 and git // 2 < NCH:
                stage_a(git // 2)
            if git % 2 == 1 and 0 <= (git - 1) // 2 < NCH:
                stage_b1((git - 1) // 2)
            if 0 <= git - 4 < NG:
                stage_c1(git - 4)


def host_prep(x, w_enc, b_enc, w_kp, b_kp):
    """Build per-core input maps (pure relayout, untimed)."""
    idx, ident = _static_tables()
    xpad = np.pad(x, ((0, 0), (0, 0), (2, 2), (2, 2)))  # [B, C, 68, 68]
    w_encT = np.ascontiguousarray(w_enc.T)              # [256, 64]
    w_kp9 = np.ascontiguousarray(
        np.transpose(w_kp, (1, 2, 3, 0)).reshape(COMP, 9 * KP)
    )
    blob = np.zeros((128, 720), np.uint8)
    wenc_bf = w_encT.astype(ml_dtypes.bfloat16).reshape(2, 128, 64)
    blob[:, 0:256] = np.ascontiguousarray(
        wenc_bf.transpose(1, 0, 2)).view(np.uint8).reshape(128, 256)
    blob[0:COMP, 256:260] = np.ascontiguousarray(
        np.asarray(b_enc, np.float32).reshape(COMP, 1)).view(np.uint8)
    blob[0:KP, 260:264] = np.ascontiguousarray(
        np.asarray(b_kp, np.float32).reshape(KP, 1)).view(np.uint8)
    blob[:, 264:520] = ident.view(np.uint8).reshape(128, 256)
    blob[:, 520:720] = idx.view(np.uint8).reshape(128, 200)

    in_maps = []
    for core in range(NCORES):
        b, q = core // 4, core % 4
        sl = xpad[b, :, 16 * q: 16 * q + RP, :]          # [C, 20, 68]
        xs_c = np.ascontiguousarray(
            sl.reshape(C, GRID)).astype(ml_dtypes.bfloat16)
        # patch tiles: [PATCH, NT, C], p = ww*6 + hh  (p-major for one big DMA)
        pat = np.empty((NT, PATCH, C), dtype=ml_dtypes.bfloat16)
        for t in range(NT):
            th, tw = t // NTW, t % NTW
            blk = sl[:, 2 * th: 2 * th + 6, TPW * tw: TPW * tw + 20]  # [C,6,20]
            pat[t] = np.transpose(blk, (2, 1, 0)).reshape(PATCH, C)
        pat = np.ascontiguousarray(np.transpose(pat, (1, 0, 2)))
        in_maps.append({
            "xs_c": xs_c,
            "patches": pat.reshape(PATCH, NT * C),
            "w_kp9": w_kp9,
            "blob": blob,
        })
    return in_maps


def host_assemble(results):
    """results: list of 8 dicts with 'out' [128, NT*C] -> full [B, C, 128, 128]."""
    out = np.empty((B, C, H * S, W * S), dtype=np.float32)
    for core in range(NCORES):
        b, q = core // 4, core % 4
        # out rows m = (2p+q2)*32 + h'*16 + w', cols (t, c)
        a = results[core]["out"].astype(np.float32).reshape(
            2, 2, TPH, TPW, NTH, NTW, C)
        # dims: p, q2, h', w', th, tw, c -> [c, th, h', p, tw, w', q2]
        o = np.transpose(a, (6, 4, 2, 0, 5, 3, 1)).reshape(C, 32, 128)
        out[b, :, 32 * q: 32 * (q + 1), :] = o
    return out


_NC_CACHE = None


def kernel(x, w_enc, b_enc, w_kp, b_kp):
    global _NC_CACHE
    x = np.asarray(x)
    w_enc = np.asarray(w_enc)
    b_enc = np.asarray(b_enc)
    w_kp = np.asarray(w_kp)
    b_kp = np.asarray(b_kp)
    if _NC_CACHE is None:
        _NC_CACHE = build_kernel()
    nc = _NC_CACHE
    in_maps = host_prep(x, w_enc, b_enc, w_kp, b_kp)
    trace = os.environ.get("CARAFE_TRACE", "0") == "1"
    res = run_bass_kernel_spmd(nc, in_maps, list(range(NCORES)), trace=trace)
    out = host_assemble(res.results)
    if trace:
        kernel.last_exec_time_ns = res.exec_time_ns
        kernel.last_results = res
    return out
